# revision 23
# baseline (speedup 1.0000x reference)
"""DirGNN (3-layer directional GCN + mean-pool + LN + MLP) on 8 Trainium2
NeuronCores.

Sharding: each core owns N/8 output nodes.  Per GCN direction the host sorts
that core's edges by segment node (dst for "in", src for "out") into windows
of WIN=256 nodes, each split into seg-aligned buckets of B=32 nodes.  Edges
of one (window, bucket, index-half) go into <=128-edge chunks (slot counts
equalized across cores so one SPMD program serves all 8), with per-window
overflow chunks absorbing the tail.  Per chunk the host builds M[edge, seg]
holding the GCN norm dinv[src]*dinv[dst] (bf16), so no per-edge scaling runs
on device.  On device: dma_gather fetches message rows (bf16, padded to
256 B), PE computes aggT[64f, segs] += msgs.T @ M into static PSUM column
ranges, layer update is feature-major matmuls with alpha-folded weights, ACT
relu + per-partition bias, PE transpose back to node-major, AllGather of bf16
shards for the next layer's gather source.  Final: pooling matmuls against
host-built 1/cnt matrices, AllReduce, LayerNorm (affine folded into P1), MLP.
"""

import math
import numpy as np
import ml_dtypes

BF16 = ml_dtypes.bfloat16


class Cfg:
    def __init__(self, N=50000, E=800000, G=64, NC=8):
        self.N, self.E, self.G, self.NC = N, E, G, NC
        self.F = 64            # features
        self.F2 = 128          # padded row width (256 B bf16)
        self.NSH = N // NC     # nodes per core
        self.WIN = 256         # psum window (nodes)
        self.B = 32            # bucket width (segs) == regular M width
        self.CAPL = 3          # max regular chunks per low-half bucket
        self.CAPH = 1          # max regular chunks per high-half bucket
        self.KWIN = 1          # windows per gather batch
        self.NPIECE = 3        # gather pieces per batch (SWDGE queue split)
        self.HALF = 32768      # int16 index split
        self.ALPHA = 0.5
        self.LN_EPS = 1e-5
        self.NWIN = math.ceil(self.NSH / self.WIN)
        self.NKB = math.ceil(self.NWIN / self.KWIN)
        self.NTP = math.ceil(self.NSH / 128)   # transpose tiles
        self.NB = math.ceil(self.NSH / 512)    # layer-matmul node batches


# ---------------------------------------------------------------------------
# host-side packing
# ---------------------------------------------------------------------------

def pack_dir(cfg, seg, gid, nrm):
    NC, NSH, WIN = cfg.NC, cfg.NSH, cfg.WIN
    B, CAPL, CAPH = cfg.B, cfg.CAPL, cfg.CAPH
    NBK = WIN // B
    NWIN, NKB, KWIN = cfg.NWIN, cfg.NKB, cfg.KWIN

    per_core_edges = []
    cnt = np.zeros((NC, NWIN, NBK, 2), np.int64)
    for c in range(NC):
        base = c * NSH
        m = (seg >= base) & (seg < base + NSH)
        sl = (seg[m] - base).astype(np.int64)
        gi = gid[m].astype(np.int64)
        nv = nrm[m].astype(np.float32)
        w = sl // WIN
        b = (sl % WIN) // B
        half = (gi >= cfg.HALF).astype(np.int64)
        order = np.lexsort((sl, b, w, half))
        sl, gi, nv, w, b, half = (a[order] for a in (sl, gi, nv, w, b, half))
        np.add.at(cnt[c], (w, b, half), 1)
        per_core_edges.append((sl, gi, nv, w, b, half))

    eq = np.ceil(cnt / 128).astype(np.int64).max(0)
    caps = np.array([CAPL, CAPH])[None, None, :]
    slots_reg = np.minimum(eq, caps)
    ovf_edges = np.maximum(cnt - slots_reg[None] * 128, 0).sum(2)
    ovf_chunks = np.ceil(ovf_edges / 128).astype(np.int64).max(0)  # [NWIN, 2]

    chunks = []
    gathers = [[] for _ in range(NKB)]
    mm = [[] for _ in range(NKB)]
    pos = nreg = novf = 0
    for kb in range(NKB):
        ws = list(range(kb * KWIN, min((kb + 1) * KWIN, NWIN)))
        for half in (0, 1):
            c0 = pos
            for w in ws:
                for b in range(NBK):
                    for _ in range(int(slots_reg[w, b, half])):
                        chunks.append(dict(kind="reg", w=w, half=half, b=b,
                                           pos=pos, ri=nreg))
                        pos += 1; nreg += 1
                for _ in range(int(ovf_chunks[w, half])):
                    chunks.append(dict(kind="ovf", w=w, half=half, b=None,
                                       pos=pos, oi=novf))
                    pos += 1; novf += 1
            if pos > c0:
                gathers[kb].append([c0, pos, half])
        for w in ws:
            for half in (0, 1):
                for ch in chunks:
                    if ch["w"] == w and ch["half"] == half:
                        mm[kb].append(ch)
    NCH = pos
    # split each kb's (c0,c1,half) ranges into ~NPIECE pieces total so they
    # can fan out across the 4 SWDGE queues (each piece gets its own SBUF
    # tile: cross-queue writes to one tile would serialize on data-landing)
    pieces = []
    for kb in range(NKB):
        pl = []
        tot = sum(c1 - c0 for (c0, c1, _h) in gathers[kb])
        if tot == 0:
            pieces.append(pl)
            continue
        tgt = max(1, math.ceil(tot / cfg.NPIECE))
        for (c0, c1, half) in gathers[kb]:
            n = c1 - c0
            np_ = max(1, round(n / tgt))
            bounds = [c0 + (n * i) // np_ for i in range(np_)] + [c1]
            for i in range(np_):
                if bounds[i + 1] > bounds[i]:
                    pl.append((bounds[i], bounds[i + 1], half))
        pieces.append(pl)
    structure = dict(NCH=NCH, NREG=nreg, NOVF=novf, gathers=gathers,
                     pieces=pieces, mm=mm)

    reg_slots_of = {}
    ovf_slots_of = {}
    for ch in chunks:
        if ch["kind"] == "reg":
            reg_slots_of.setdefault((ch["w"], ch["b"], ch["half"]), []).append(ch)
        else:
            ovf_slots_of.setdefault((ch["w"], ch["half"]), []).append(ch)

    per_core = []
    for c in range(NC):
        sl, gi, nv, w, b, half = per_core_edges[c]
        idx_flat = np.zeros(NCH * 128, np.int16)
        Mreg = np.zeros((nreg, 128, B), np.float32)
        Movf = np.zeros((max(novf, 1), 128, WIN), np.float32)
        key = (half * NWIN + w) * NBK + b
        bounds = np.flatnonzero(np.diff(key)) + 1
        starts = np.concatenate([[0], bounds]) if len(sl) else []
        ends = np.concatenate([bounds, [len(sl)]]) if len(sl) else []
        ovf_pool = {}
        for s, e in zip(starts, ends):
            wi, bi, hi = int(w[s]), int(b[s]), int(half[s])
            slots = reg_slots_of.get((wi, bi, hi), [])
            take = min(e - s, len(slots) * 128)
            for k, ch in enumerate(slots):
                a0 = s + k * 128
                a1 = min(a0 + 128, s + take)
                if a1 <= a0:
                    break
                ln = a1 - a0
                p0 = ch["pos"] * 128
                idx_flat[p0:p0 + ln] = (gi[a0:a1] - (cfg.HALF if hi else 0)).astype(np.int16)
                Mreg[ch["ri"], np.arange(ln), sl[a0:a1] - wi * WIN - bi * B] = nv[a0:a1]
            if take < e - s:
                ovf_pool.setdefault((wi, hi), []).append(np.arange(s + take, e))
        for (wi, hi), lists in ovf_pool.items():
            ee = np.concatenate(lists)
            slots = ovf_slots_of[(wi, hi)]
            assert len(ee) <= len(slots) * 128
            for k, ch in enumerate(slots):
                a = ee[k * 128:(k + 1) * 128]
                if len(a) == 0:
                    break
                p0 = ch["pos"] * 128
                idx_flat[p0:p0 + len(a)] = (gi[a] - (cfg.HALF if hi else 0)).astype(np.int16)
                Movf[ch["oi"], np.arange(len(a)), sl[a] - wi * WIN] = nv[a]
        idx_w = np.zeros((128, NCH * 8), np.int16)
        for kb in range(NKB):
            for (c0, c1, hf) in gathers[kb]:
                blk = idx_flat[c0 * 128: c1 * 128].reshape(-1, 16)
                idx_w[:16, c0 * 8: c1 * 8] = blk.T
        idx_w[16:32] = idx_w[:16]
        idx_w[32:64] = idx_w[:32]
        idx_w[64:128] = idx_w[:64]
        per_core.append(dict(
            idx=idx_w,
            M=np.ascontiguousarray(Mreg.transpose(1, 0, 2)).astype(BF16),
            Movf=np.ascontiguousarray(Movf.transpose(1, 0, 2)).astype(BF16),
        ))
    return structure, per_core


def host_prep(cfg, inputs):
    N, G, F = cfg.N, cfg.G, cfg.F
    edge_src = np.asarray(inputs["edge_src"]).astype(np.int64)
    edge_dst = np.asarray(inputs["edge_dst"]).astype(np.int64)
    batch = np.asarray(inputs["batch"]).astype(np.int64)
    ar = np.arange(N, dtype=np.int64)
    src = np.concatenate([edge_src, ar])
    dst = np.concatenate([edge_dst, ar])
    deg_in = np.bincount(dst, minlength=N).astype(np.float32)
    deg_out = np.bincount(src, minlength=N).astype(np.float32)
    dinv_in = np.where(deg_in > 0, 1.0 / np.sqrt(deg_in), 0.0).astype(np.float32)
    dinv_out = np.where(deg_out > 0, 1.0 / np.sqrt(deg_out), 0.0).astype(np.float32)
    # self-loop edges are handled on-device via a PSUM preload of
    # s_d[n] * h[n]; only the real edges go through the gather machinery
    es, ed = edge_src, edge_dst
    norm_in = dinv_in[es] * dinv_in[ed]
    norm_out = dinv_out[es] * dinv_out[ed]

    st_in, pc_in = pack_dir(cfg, ed, es, norm_in)
    st_out, pc_out = pack_dir(cfg, es, ed, norm_out)

    x = np.asarray(inputs["x"], np.float32)
    xpad = np.zeros((N, cfg.F2), BF16)
    xpad[:, :F] = x.astype(BF16)

    NSHP = cfg.NWIN * cfg.WIN
    s_in = dinv_in * dinv_in     # weight of the self edge (n, n), per dir
    s_out = dinv_out * dinv_out
    S_cores, xT0_cores = [], []
    for c in range(cfg.NC):
        sh = slice(c * cfg.NSH, (c + 1) * cfg.NSH)
        S2 = np.zeros((2, F, NSHP), BF16)
        S2[0, :, :cfg.NSH] = np.broadcast_to(s_in[sh][None, :], (F, cfg.NSH)).astype(BF16)
        S2[1, :, :cfg.NSH] = np.broadcast_to(s_out[sh][None, :], (F, cfg.NSH)).astype(BF16)
        S_cores.append(S2)
        xt = np.zeros((F, NSHP), BF16)
        xt[:, :cfg.NSH] = x[sh].T.astype(BF16)
        xT0_cores.append(xt)

    wmat = np.zeros((F, 6, F), np.float32)
    bvec = np.zeros((F, 3), np.float32)
    for li, l in enumerate((1, 2, 3)):
        wmat[:, 2 * li + 0] = cfg.ALPHA * np.asarray(inputs[f"W{l}_out"], np.float32)
        wmat[:, 2 * li + 1] = (1 - cfg.ALPHA) * np.asarray(inputs[f"W{l}_in"], np.float32)
        bvec[:, li] = (cfg.ALPHA * np.asarray(inputs[f"b{l}_out"], np.float32)
                       + (1 - cfg.ALPHA) * np.asarray(inputs[f"b{l}_in"], np.float32))
    wmat = wmat.astype(BF16)

    cntg = np.bincount(batch, minlength=G).astype(np.float32)
    pw = 1.0 / np.maximum(cntg, 1.0)
    Pn_cores = []
    for c in range(cfg.NC):
        Pn = np.zeros((128, cfg.NTP, G), np.float32)
        for t in range(cfg.NTP):
            n0 = c * cfg.NSH + t * 128
            ln = min(128, (c + 1) * cfg.NSH - n0)
            nodes = np.arange(n0, n0 + ln)
            Pn[np.arange(ln), t, batch[nodes]] = pw[batch[nodes]]
        Pn_cores.append(Pn.astype(BF16))

    ln_w = np.asarray(inputs["ln_w"], np.float32)
    ln_b = np.asarray(inputs["ln_b"], np.float32)
    P1w = np.asarray(inputs["P1_w"], np.float32)
    P1b = np.asarray(inputs["P1_b"], np.float32)
    P2w = np.asarray(inputs["P2_w"], np.float32)
    P2b = np.asarray(inputs["P2_b"], np.float32)

    shared = dict(
        xpad=xpad, wmat=wmat, bvec=bvec,
        p1w=ln_w[:, None] * P1w,
        p1b=(P1b + ln_b @ P1w)[:, None],
        p2w=P2w, p2b=P2b[:, None],
        ident_bf=np.eye(F, dtype=BF16),
        ident_f32=np.eye(F, dtype=np.float32),
        epsb=np.full((G, 1), cfg.LN_EPS, np.float32),
    )
    in_maps = []
    for c in range(cfg.NC):
        m = dict(shared)
        for d, pc in (("in", pc_in), ("out", pc_out)):
            m[f"idx_{d}"] = pc[c]["idx"]
            m[f"M_{d}"] = pc[c]["M"]
            if max(st_in["NOVF"] if d == "in" else st_out["NOVF"], 0) > 0:
                m[f"Movf_{d}"] = pc[c]["Movf"]
        m["Pn"] = Pn_cores[c]
        m["Sdir"] = S_cores[c]
        m["xT0"] = xT0_cores[c]
        in_maps.append(m)
    return (st_in, st_out), in_maps


# ---------------------------------------------------------------------------
# device program
# ---------------------------------------------------------------------------

def build_program(cfg, st_in, st_out, stage="full", rep_count=1):
    import concourse.bass as bass
    import concourse.mybir as mybir
    import concourse.bacc as bacc
    import concourse.tile as tile
    import contextlib

    F, F2, G = cfg.F, cfg.F2, cfg.G
    NSH, WIN, B = cfg.NSH, cfg.WIN, cfg.B
    NWIN, NKB, NTP, NB = cfg.NWIN, cfg.NKB, cfg.NTP, cfg.NB
    KWIN = cfg.KWIN
    bf = mybir.dt.bfloat16
    f32 = mybir.dt.float32
    i16 = mybir.dt.int16
    AF = mybir.ActivationFunctionType

    nc = bacc.Bacc(None, target_bir_lowering=False, num_devices=cfg.NC,
                   num_swdge_queues=4)
    sts = {"in": st_in, "out": st_out}

    dts = {}
    dts["xpad"] = nc.dram_tensor("xpad", [cfg.N, F2], bf, kind="ExternalInput")
    for d in ("in", "out"):
        st = sts[d]
        dts[f"idx_{d}"] = nc.dram_tensor(f"idx_{d}", [128, st["NCH"] * 8], i16,
                                         kind="ExternalInput")
        dts[f"M_{d}"] = nc.dram_tensor(f"M_{d}", [128, st["NREG"], B], bf,
                                       kind="ExternalInput")
        if st["NOVF"]:
            dts[f"Movf_{d}"] = nc.dram_tensor(f"Movf_{d}", [128, st["NOVF"], WIN],
                                              bf, kind="ExternalInput")
    NSHP_ = cfg.NWIN * cfg.WIN
    dts["Sdir"] = nc.dram_tensor("Sdir", [2, F, NSHP_], bf, kind="ExternalInput")
    dts["xT0"] = nc.dram_tensor("xT0", [F, NSHP_], bf, kind="ExternalInput")
    dts["wmat"] = nc.dram_tensor("wmat", [F, 6, F], bf, kind="ExternalInput")
    dts["bvec"] = nc.dram_tensor("bvec", [F, 3], f32, kind="ExternalInput")
    dts["Pn"] = nc.dram_tensor("Pn", [128, NTP, G], bf, kind="ExternalInput")
    dts["p1w"] = nc.dram_tensor("p1w", [F, 128], f32, kind="ExternalInput")
    dts["p1b"] = nc.dram_tensor("p1b", [128, 1], f32, kind="ExternalInput")
    dts["p2w"] = nc.dram_tensor("p2w", [128, 2], f32, kind="ExternalInput")
    dts["p2b"] = nc.dram_tensor("p2b", [2, 1], f32, kind="ExternalInput")
    dts["ident_bf"] = nc.dram_tensor("ident_bf", [F, F], bf, kind="ExternalInput")
    dts["ident_f32"] = nc.dram_tensor("ident_f32", [F, F], f32, kind="ExternalInput")
    dts["epsb"] = nc.dram_tensor("epsb", [G, 1], f32, kind="ExternalInput")
    out_dram = nc.dram_tensor("out", [2, G], f32, kind="ExternalOutput")

    with tile.TileContext(nc) as tc:
        ctx = contextlib.ExitStack()
        with ctx:
            const = ctx.enter_context(tc.tile_pool(name="const", bufs=1))
            sb_idx = ctx.enter_context(tc.tile_pool(name="sbidx", bufs=1))
            sb_m = ctx.enter_context(tc.tile_pool(name="sbm", bufs=4))
            sb_msg = ctx.enter_context(tc.tile_pool(name="sbmsg", bufs=16))
            sb_agg = ctx.enter_context(tc.tile_pool(name="sbagg", bufs=1))
            sb_h = ctx.enter_context(tc.tile_pool(name="sbh", bufs=2))
            sb_big = ctx.enter_context(tc.tile_pool(name="sbbig", bufs=1))
            ps_layer = ctx.enter_context(tc.tile_pool(name="pslayer", bufs=2,
                                                      space="PSUM"))
            ps_tr = ctx.enter_context(tc.tile_pool(name="pstr", bufs=2,
                                                   space="PSUM"))
            dram = ctx.enter_context(tc.tile_pool(name="dram", bufs=2, space="DRAM"))

            wmat_t = const.tile([F, 6, F], bf)
            nc.sync.dma_start(wmat_t[:], dts["wmat"][:])
            bvec_t = const.tile([F, 3], f32)
            nc.sync.dma_start(bvec_t[:], dts["bvec"][:])
            ident_bf_t = const.tile([F, F], bf)
            nc.sync.dma_start(ident_bf_t[:], dts["ident_bf"][:])
            ident_f32_t = const.tile([F, F], f32)
            nc.sync.dma_start(ident_f32_t[:], dts["ident_f32"][:])
            epsb_t = const.tile([G, 1], f32)
            nc.sync.dma_start(epsb_t[:], dts["epsb"][:])
            Pn_t = const.tile([128, NTP, G], bf)
            nc.sync.dma_start(Pn_t[:], dts["Pn"][:])
            p1w_t = const.tile([F, 128], f32)
            nc.sync.dma_start(p1w_t[:], dts["p1w"][:])
            p1b_t = const.tile([128, 1], f32)
            nc.sync.dma_start(p1b_t[:], dts["p1b"][:])
            p2w_t = const.tile([128, 2], f32)
            nc.sync.dma_start(p2w_t[:], dts["p2w"][:])
            p2b_t = const.tile([2, 1], f32)
            nc.sync.dma_start(p2b_t[:], dts["p2b"][:])

            idx_t = {}
            for d in ("in", "out"):
                NCH = sts[d]["NCH"]
                idx_t[d] = sb_idx.tile([128, NCH * 8], i16, tag=f"idx{d}",
                                       name=f"idx{d}")
                nc.sync.dma_start(idx_t[d][:], dts[f"idx_{d}"][:])

            NSHP = NWIN * WIN
            aggT = {d: sb_agg.tile([F, NSHP], bf, tag=f"agg{d}", name=f"agg{d}")
                    for d in ("in", "out")}

            S_t = {}
            for di, d in enumerate(("in", "out")):
                S_t[d] = const.tile([F, NSHP], bf, name=f"S{d}")
                nc.sync.dma_start(S_t[d][:], dts["Sdir"][di])
            xT0_t = const.tile([F, NSHP], bf, name="xT0")
            nc.sync.dma_start(xT0_t[:], dts["xT0"][:])

            keep_t = const.tile([128, F2], bf, name="keep")

            PMAX = max((p1 - p0)
                       for st in sts.values()
                       for pl in st["pieces"] for (p0, p1, _h) in pl)

            def _kb_extent(st, kind, key):
                vals = [0]
                for kb in range(NKB):
                    ids = [ch[key] for ch in st["mm"][kb] if ch["kind"] == kind]
                    if ids:
                        vals.append(max(ids) - min(ids) + 1)
                return max(vals)
            MRMAX = max(_kb_extent(st, "reg", "ri") for st in sts.values()) or 1
            MOMAX = max(_kb_extent(st, "ovf", "oi") for st in sts.values()) or 1

            qctr = [0]
            hT = sb_big.tile([F, NSHP], bf, tag="hT", name="hT")
            hn_t = sb_big.tile([128, NTP, F], bf, tag="hn", name="hn")

            def kb_aggregate(d, kb, src_dram, hsrc_fm, ps_agg):
                """One gather batch of one direction: gather pieces (4 SWDGE
                queues), per-chunk matmuls into PSUM windows preloaded with the
                self-loop term s_d[n]*h[n], flush windows into aggT."""
                st = sts[d]
                plist = st["pieces"][kb]
                if not plist:
                    return
                do_gather = not stage.endswith("mm")
                do_mm = not stage.endswith("gth")
                ptiles = []   # (p0, p1, tile)
                for pi, (p0, p1, half) in enumerate(plist):
                    in_ap = src_dram[cfg.HALF:, :] if half else src_dram[:]
                    pe = p1 if do_gather else p0 + 1
                    n_idx = (pe - p0) * 128
                    mt = sb_msg.tile([128, PMAX, F2], bf, tag="msgs",
                                     name=f"msgs{pi}")
                    nc.gpsimd.dma_gather(
                        out_ap=mt[:, : pe - p0, :],
                        in_ap=in_ap,
                        idxs_ap=idx_t[d][:, p0 * 8: pe * 8],
                        num_idxs=n_idx,
                        num_idxs_reg=n_idx,
                        elem_size=F2,
                        single_packet=False,
                        queue_num=qctr[0] % 4,
                    )
                    qctr[0] += 1
                    ptiles.append((p0, pe, mt))
                    if not do_gather:
                        break
                if not do_mm:
                    nc.vector.tensor_copy(keep_t[:], ptiles[0][2][:, 0, :])
                    return

                def chunk_lhs(pos):
                    for (p0, p1, mt) in ptiles:
                        if p0 <= pos < p1:
                            return mt[:, pos - p0, :F]
                    raise AssertionError(pos)

                mmk = st["mm"][kb]
                ris = [ch["ri"] for ch in mmk if ch["kind"] == "reg"]
                ois = [ch["oi"] for ch in mmk if ch["kind"] == "ovf"]
                r0, r1 = (min(ris), max(ris) + 1) if ris else (0, 0)
                o0, o1 = (min(ois), max(ois) + 1) if ois else (0, 0)
                M_kb = Mo_kb = None
                if r1 > r0:
                    M_kb = sb_m.tile([128, MRMAX, B], bf, tag="M", name="Mkb")
                    nc.sync.dma_start(M_kb[:, : r1 - r0, :],
                                      dts[f"M_{d}"][:, r0:r1, :])
                if o1 > o0:
                    Mo_kb = sb_m.tile([128, MOMAX, WIN], bf, tag="Mo",
                                      name="Mokb")
                    nc.sync.dma_start(Mo_kb[:, : o1 - o0, :],
                                      dts[f"Movf_{d}"][:, o0:o1, :])
                cur_w = None
                pt = None
                def flush(w):
                    wlen = min(WIN, NSH - w * WIN)
                    nc.scalar.activation(
                        aggT[d][:, w * WIN: w * WIN + wlen],
                        pt[:, :wlen], AF.Copy)
                for ch in mmk:
                    if ch["w"] != cur_w:
                        if cur_w is not None:
                            flush(cur_w)
                        cur_w = ch["w"]
                        pt = ps_agg.tile([F, WIN], f32, tag=f"pw{d}",
                                         name=f"pw{d}")
                        # self-loop term: agg[n] starts at s_d[n]*h[n]
                        wlen = min(WIN, NSH - cur_w * WIN)
                        nc.vector.tensor_mul(
                            pt[:, :wlen],
                            hsrc_fm[:, cur_w * WIN: cur_w * WIN + wlen],
                            S_t[d][:, cur_w * WIN: cur_w * WIN + wlen])
                    if ch["kind"] == "reg":
                        nc.tensor.matmul(
                            pt[:, ch["b"] * B:(ch["b"] + 1) * B],
                            chunk_lhs(ch["pos"]), M_kb[:, ch["ri"] - r0, :],
                            start=False, stop=False, skip_group_check=True)
                    else:
                        nc.tensor.matmul(
                            pt[:], chunk_lhs(ch["pos"]), Mo_kb[:, ch["oi"] - o0, :],
                            start=False, stop=False, skip_group_check=True)
                if cur_w is not None:
                    flush(cur_w)

            def update_windows(layer, w0, w1):
                """h = act(W_out^T agg_out + W_in^T agg_in + b) for node columns
                of windows [w0, w1), then transpose to node-major hn tiles."""
                li = layer - 1
                act = AF.Relu if layer < 3 else AF.Identity
                n0 = w0 * WIN
                ln = min(w1 * WIN, NSH) - n0
                if ln <= 0:
                    return
                pb = ps_layer.tile([F, WIN], f32, tag="lay", name="lay")
                nc.tensor.matmul(pb[:, :ln], wmat_t[:, 2 * li + 0, :],
                                 aggT["out"][:, n0:n0 + ln],
                                 start=True, stop=False)
                nc.tensor.matmul(pb[:, :ln], wmat_t[:, 2 * li + 1, :],
                                 aggT["in"][:, n0:n0 + ln],
                                 start=False, stop=True)
                nc.scalar.activation(hT[:, n0:n0 + ln], pb[:, :ln],
                                     act, bias=bvec_t[:, li:li + 1])
                for t in range(w0 * (WIN // 128), min(w1 * (WIN // 128), NTP)):
                    tn0 = t * 128
                    tln = min(128, NSH - tn0)
                    ptt = ps_tr.tile([128, F], bf, tag="tr", name="tr")
                    nc.tensor.transpose(ptt[:tln, :], hT[:, tn0:tn0 + tln],
                                        ident_bf_t)
                    nc.vector.tensor_copy(hn_t[:tln, t, :], ptt[:tln, :])

            def store_windows(shard, w0, w1):
                t0 = w0 * (WIN // 128)
                t1 = min(w1 * (WIN // 128), NTP)
                tfull = t1 - (1 if (t1 == NTP and NSH % 128) else 0)
                if tfull > t0:
                    nc.sync.dma_start(
                        shard[t0 * 128: tfull * 128, :].rearrange(
                            "(t p) f -> p t f", p=128)[:, :, :F],
                        hn_t[:, t0:tfull, :])
                if t1 == NTP and NSH % 128:
                    nc.sync.dma_start(shard[tfull * 128: NSH, :F],
                                      hn_t[: NSH % 128, tfull, :])

            def bail():
                logits = const.tile([2, G], f32, name="bail")
                nc.vector.memset(logits[:], 0.0)
                nc.sync.dma_start(out_dram[:], logits[:])

            for _rep in range(rep_count):
                hfull_prev = None
                for layer in (1, 2, 3):
                    src_dram = dts["xpad"][:] if layer == 1 else hfull_prev[:]
                    hsrc_fm = xT0_t if layer == 1 else hT
                    shard = None
                    if layer < 3:
                        shard = dram.tile([NSH, F2], bf, tag="shard",
                                          name="shard")
                    with tc.tile_pool(name=f"psi{layer}r{_rep}", bufs=2,
                                      space="PSUM") as ps_in, \
                         tc.tile_pool(name=f"pso{layer}r{_rep}", bufs=2,
                                      space="PSUM") as ps_out:
                        for kb in range(NKB):
                            kb_aggregate("in", kb, src_dram, hsrc_fm, ps_in)
                            kb_aggregate("out", kb, src_dram, hsrc_fm, ps_out)
                            if stage in (f"{layer}agg", f"{layer}gth",
                                         f"{layer}mm"):
                                continue
                            w0, w1 = kb * KWIN, min((kb + 1) * KWIN, NWIN)
                            update_windows(layer, w0, w1)
                            if layer < 3:
                                store_windows(shard, w0, w1)
                    if stage in (f"{layer}agg", f"{layer}gth", f"{layer}mm",
                                 f"{layer}upd"):
                        bail(); break
                    if layer < 3:
                        full = dram.tile([cfg.N, F2], bf, tag="hfull",
                                         name="hfull")
                        nc.gpsimd.collective_compute(
                            "AllGather", mybir.AluOpType.bypass,
                            replica_groups=[list(range(cfg.NC))],
                            ins=[shard.opt()], outs=[full.opt()],
                        )
                        hfull_prev = full
                        if stage == f"{layer}col":
                            bail(); break
                    else:
                        hn3 = hn_t

                do_final = stage == "full"
                if do_final:
                  with tc.tile_pool(name=f"pssm{_rep}", bufs=1, space="PSUM") as ps_sm:
                      pp = ps_sm.tile([F, G], f32, tag="pool", name="pool")
                      for t in range(NTP):
                          ln = min(128, NSH - t * 128)
                          nc.tensor.matmul(pp[:], hn3[:ln, t, :], Pn_t[:ln, t, :],
                                           start=(t == 0), stop=(t == NTP - 1))
                      pooledT_part = const.tile([F, G], f32)
                      nc.scalar.activation(pooledT_part[:], pp[:], AF.Copy)
                      bounce_in = dram.tile([F, G], f32, tag="cin", name="cin")
                      bounce_out = dram.tile([F, G], f32, tag="cout", name="cout")
                      nc.gpsimd.dma_start(bounce_in[:], pooledT_part[:])
                      nc.gpsimd.collective_compute(
                          "AllReduce", mybir.AluOpType.add,
                          replica_groups=[list(range(cfg.NC))],
                          ins=[bounce_in.opt()], outs=[bounce_out.opt()],
                      )
                      pooledT = const.tile([F, G], f32)
                      nc.sync.dma_start(pooledT[:], bounce_out[:])

                      ptr = ps_sm.tile([G, F], f32, tag="lntr", name="lntr")
                      nc.tensor.transpose(ptr[:], pooledT[:], ident_f32_t[:])
                      z = const.tile([G, F], f32)
                      nc.vector.tensor_copy(z[:], ptr[:])
                      zsum = const.tile([G, 1], f32)
                      nc.vector.tensor_reduce(zsum[:], z[:], mybir.AxisListType.X,
                                              mybir.AluOpType.add)
                      zmean = const.tile([G, 1], f32)
                      nc.scalar.activation(zmean[:], zsum[:], AF.Copy, scale=1.0 / F)
                      zc = const.tile([G, F], f32)
                      nc.vector.tensor_scalar_sub(zc[:], z[:], zmean[:])
                      zsq = const.tile([G, F], f32)
                      nc.vector.tensor_mul(zsq[:], zc[:], zc[:])
                      ssum = const.tile([G, 1], f32)
                      nc.vector.tensor_reduce(ssum[:], zsq[:], mybir.AxisListType.X,
                                              mybir.AluOpType.add)
                      std = const.tile([G, 1], f32)
                      nc.scalar.activation(std[:], ssum[:], AF.Sqrt,
                                           scale=1.0 / F, bias=epsb_t[:])
                      rstd = const.tile([G, 1], f32)
                      nc.vector.reciprocal(rstd[:], std[:])
                      zn = const.tile([G, F], f32)
                      nc.vector.tensor_scalar_mul(zn[:], zc[:], rstd[:])

                      ptr2 = ps_sm.tile([F, G], f32, tag="lntr", name="lntr2")
                      nc.tensor.transpose(ptr2[:], zn[:], ident_f32_t[:])
                      znT = const.tile([F, G], f32)
                      nc.vector.tensor_copy(znT[:], ptr2[:])
                      pm1 = ps_sm.tile([128, G], f32, tag="mlp1", name="mlp1")
                      nc.tensor.matmul(pm1[:], p1w_t[:], znT[:], start=True, stop=True)
                      a1 = const.tile([128, G], f32)
                      nc.scalar.activation(a1[:], pm1[:], AF.Relu, bias=p1b_t[:])
                      pm2 = ps_sm.tile([2, G], f32, tag="mlp2", name="mlp2")
                      nc.tensor.matmul(pm2[:], p2w_t[:], a1[:], start=True, stop=True)
                      logits = const.tile([2, G], f32)
                      nc.scalar.activation(logits[:], pm2[:], AF.Identity, bias=p2b_t[:])
                      nc.sync.dma_start(out_dram[:], logits[:])

    nc.compile()
    return nc


# ---------------------------------------------------------------------------
# entry point
# ---------------------------------------------------------------------------

_CACHE = {}


def _run(cfg, inputs, trace=False):
    from concourse import bass_utils
    (st_in, st_out), in_maps = host_prep(cfg, inputs)
    key = (cfg.N, cfg.E,
           st_in["NCH"], st_out["NCH"], st_in["NOVF"], st_out["NOVF"],
           tuple(ch["pos"] for ch in st_in["mm"][0][:50]))
    if key not in _CACHE:
        _CACHE[key] = build_program(cfg, st_in, st_out)
    nc = _CACHE[key]
    r = bass_utils.run_bass_kernel_spmd(nc, in_maps,
                                        core_ids=list(range(cfg.NC)),
                                        trace=trace)
    out = r.results[0]["out"]
    return np.ascontiguousarray(out.T.astype(np.float32)), r


def kernel(**inputs):
    cfg = Cfg(N=50000, E=800000, G=64, NC=8)
    out, _ = _run(cfg, inputs)
    return out

